# revision 1
# baseline (speedup 1.0000x reference)
"""Trainium2 Bass kernel for the GNO (Galerkin-type linear attention) model.

Reference computation per batch element b (N=4096 tokens, d=64):
    h = x @ lift_w + lift_b
    for each of 4 layers:
        q = h@q_w+q_b ; k = h@k_w+k_b ; v = h@v_w+v_b
        kern     = (q @ k^T) / sqrt(d)          # [N, N], no softmax!
        integral = (kern @ v) / N               # [N, d]
        h        = gelu(h@blk_w+blk_b + integral)
    out = h @ proj_w + proj_b

Because there is no softmax, (q k^T) v == q (k^T v), so each layer only
needs the tiny [64,64] moment matrix ktv = k^T v.  Further,
    integral = q @ (s*ktv)  = H_aug^T (q_w_aug @ (s*ktv))
    wh + integral           = H_aug^T (blk_w_aug + q_w_aug @ (s*ktv))
with H_aug = [h^T; 1] (a ones row folds every bias into the weights), so
the whole layer update is ONE [65,64] combined weight matmul + gelu.

Sharding: batch is 2 -> data-parallel on 2 NeuronCores, one batch element
per core, everything resident in SBUF.  Sequence-sharding wider would need
a per-layer AllReduce of ktv whose ~10us latency floor exceeds the whole
per-layer compute, so it loses.
"""

import os
import sys

for _p in ("/opt/trn_rl_repo", "/root/.axon_site/_ro/trn_rl_repo"):
    if os.path.isdir(_p) and _p not in sys.path:
        sys.path.append(_p)

import numpy as np

N = 4096          # tokens per batch element (64*64)
D = 64            # hidden
DA = D + 1        # hidden + ones row
L = 4             # layers
B = 2             # batch / cores used
SCALE = (1.0 / np.sqrt(np.float32(D))) / np.float32(N)

_CACHE = {}


def _build_nc():
    """Build + compile the per-core Bass program (identical on both cores)."""
    import concourse.bass as bass
    import concourse.tile as tile
    from concourse import bacc, mybir

    f32 = mybir.dt.float32
    ts = bass.ts
    GELU = mybir.ActivationFunctionType.Gelu

    nc = bacc.Bacc("TRN2", target_bir_lowering=False, debug=False, num_devices=B)

    xt_d = nc.dram_tensor("xt", [4, N], f32, kind="ExternalInput")
    lift_d = nc.dram_tensor("liftw", [4, DA], f32, kind="ExternalInput")
    kvw_d = nc.dram_tensor("kvw", [DA, L * 2 * D], f32, kind="ExternalInput")
    qts_d = nc.dram_tensor("qts", [D, L * DA], f32, kind="ExternalInput")
    blkw_d = nc.dram_tensor("blkw", [DA, L * D], f32, kind="ExternalInput")
    proj_d = nc.dram_tensor("projw", [DA, 1], f32, kind="ExternalInput")
    y_d = nc.dram_tensor("y", [1, N], f32, kind="ExternalOutput")

    PS = bass.MemorySpace.PSUM

    with tile.TileContext(nc) as tc:
        with (
            tc.tile_pool(name="consts", bufs=1) as consts,
            tc.tile_pool(name="hbuf", bufs=1) as hbuf,
            tc.tile_pool(name="kvsb", bufs=3) as kvsb,
            tc.tile_pool(name="small", bufs=2) as small,
            tc.tile_pool(name="ps_kv", bufs=2, space=PS) as ps_kv,
            tc.tile_pool(name="ps_sm", bufs=2, space=PS) as ps_sm,
            tc.tile_pool(name="ps_up", bufs=2, space=PS) as ps_up,
        ):
            # ---- load everything into SBUF -------------------------------
            xt = consts.tile([4, N], f32, tag="xt")
            nc.sync.dma_start(xt[:], xt_d.ap())
            liftw = consts.tile([4, DA], f32, tag="liftw")
            nc.sync.dma_start(liftw[:], lift_d.ap())
            kvw = consts.tile([DA, L * 2 * D], f32, tag="kvw")
            nc.sync.dma_start(kvw[:], kvw_d.ap())
            qts = consts.tile([D, L * DA], f32, tag="qts")
            nc.sync.dma_start(qts[:], qts_d.ap())
            blkw = consts.tile([DA, L * D], f32, tag="blkw")
            nc.sync.dma_start(blkw[:], blkw_d.ap())
            projw = consts.tile([DA, 1], f32, tag="projw")
            nc.sync.dma_start(projw[:], proj_d.ap())

            # two ping-pong H_aug buffers, [65, 4096] each
            H0 = hbuf.tile([DA, N], f32, tag="h0")
            H1 = hbuf.tile([DA, N], f32, tag="h1")
            # H1's ones row never gets written by the layer loop; seed it
            # from the ones row of x^T_aug.  H0's comes from the lift matmul.
            nc.sync.dma_start(H1[D : D + 1, :], xt_d.ap()[3:4, :])

            # ---- lift: H0 = lift_aug^T @ xt  ([65, 4096]) ----------------
            for c in range(8):
                ps = ps_up.tile([DA, 512], f32, tag="up")
                nc.tensor.matmul(ps[:], liftw[:], xt[:, ts(c, 512)],
                                 start=True, stop=True)
                nc.vector.tensor_copy(H0[:, ts(c, 512)], ps[:])

            # ---- layers --------------------------------------------------
            for l in range(L):
                cur = H0 if l % 2 == 0 else H1
                nxt = H1 if l % 2 == 0 else H0
                kvw_l = kvw[:, l * 2 * D : (l + 1) * 2 * D]

                ktv_ps = ps_sm.tile([D, D], f32, tag="sm")
                for j in range(8):
                    kv_ps = ps_kv.tile([128, 512], f32, tag="kv")
                    for k in range(4):
                        t = 4 * j + k
                        # KV_nat tile [128, 128] = H_chunk^T @ kvw_aug
                        nc.tensor.matmul(kv_ps[:, ts(k, 128)],
                                         cur[:, ts(t, 128)], kvw_l,
                                         start=True, stop=True)
                    kv_sb = kvsb.tile([128, 512], f32, tag="kvt")
                    nc.vector.tensor_copy(kv_sb[:], kv_ps[:])
                    for k in range(4):
                        first = (j == 0 and k == 0)
                        last = (j == 7 and k == 3)
                        # ktv += K_tile^T @ V_tile
                        nc.tensor.matmul(ktv_ps[:],
                                         kv_sb[:, k * 128 : k * 128 + 64],
                                         kv_sb[:, k * 128 + 64 : k * 128 + 128],
                                         start=first, stop=last)

                ktv_sb = small.tile([D, D], f32, tag="ktv")
                nc.vector.tensor_copy(ktv_sb[:], ktv_ps[:])

                # W_upd = blk_w_aug + q_w_aug*s @ ktv   ([65, 64])
                weff_ps = ps_sm.tile([DA, D], f32, tag="sm")
                nc.tensor.matmul(weff_ps[:], qts[:, l * DA : (l + 1) * DA],
                                 ktv_sb[:], start=True, stop=True)
                wupd_sb = small.tile([DA, D], f32, tag="wupd")
                nc.vector.tensor_add(wupd_sb[:], weff_ps[:],
                                     blkw[:, l * D : (l + 1) * D])

                # h' = gelu(H_aug^T @ W_upd), written transposed into nxt
                for c in range(4):
                    up_ps = ps_up.tile([D, 1024], f32, tag="up")
                    for i in range(2):
                        nc.tensor.matmul(
                            up_ps[:, ts(i, 512)], wupd_sb[:],
                            cur[:, 1024 * c + 512 * i : 1024 * c + 512 * (i + 1)],
                            start=True, stop=True)
                    nc.scalar.activation(nxt[0:D, ts(c, 1024)], up_ps[:], GELU)

            # ---- proj: y = proj_aug^T @ H_final  ([1, 4096]) -------------
            Hf = H0 if L % 2 == 0 else H1
            out_sb = consts.tile([1, N], f32, tag="out")
            for c in range(8):
                pr_ps = ps_sm.tile([1, 512], f32, tag="sm")
                nc.tensor.matmul(pr_ps[:], projw[:], Hf[:, ts(c, 512)],
                                 start=True, stop=True)
                nc.vector.tensor_copy(out_sb[0:1, ts(c, 512)], pr_ps[:])
            nc.sync.dma_start(y_d.ap(), out_sb[:])

    nc.compile()
    return nc


def _prep_inputs(x, lift_w, lift_b, blk_w, blk_b, q_w, q_b, k_w, k_b, v_w,
                 v_b, proj_w, proj_b):
    """Host-side weight packing (tiny [64,64] reshuffles, negligible cost)."""
    f = lambda a: np.asarray(a, dtype=np.float32)
    x = f(x)
    lift_w, lift_b = f(lift_w), f(lift_b)
    blk_w, blk_b = f(blk_w), f(blk_b)
    q_w, q_b, k_w, k_b, v_w, v_b = f(q_w), f(q_b), f(k_w), f(k_b), f(v_w), f(v_b)
    proj_w, proj_b = f(proj_w), f(proj_b)

    lift_aug = np.zeros((4, DA), np.float32)
    lift_aug[:3, :D] = lift_w
    lift_aug[3, :D] = lift_b
    lift_aug[3, D] = 1.0  # makes the lift matmul emit H0's ones row

    kvw = np.concatenate(
        [np.concatenate([np.vstack([k_w[l], k_b[l][None]]),
                         np.vstack([v_w[l], v_b[l][None]])], axis=1)
         for l in range(L)], axis=1).astype(np.float32)          # [65, 512]
    qts = np.concatenate(
        [(np.vstack([q_w[l], q_b[l][None]]) * SCALE).T
         for l in range(L)], axis=1).astype(np.float32)          # [64, 260]
    blkw = np.concatenate(
        [np.vstack([blk_w[l], blk_b[l][None]]) for l in range(L)],
        axis=1).astype(np.float32)                               # [65, 256]
    proj = np.vstack([proj_w, proj_b[None]]).astype(np.float32)  # [65, 1]

    in_maps = []
    for b in range(B):
        xt = np.concatenate([x[b].reshape(N, 3).T,
                             np.ones((1, N), np.float32)], axis=0)
        in_maps.append({"xt": np.ascontiguousarray(xt), "liftw": lift_aug,
                        "kvw": kvw, "qts": qts, "blkw": blkw, "projw": proj})
    return in_maps, x.shape


def _get_runner():
    """Compile once, return a fn(in_maps) -> list[{name: np.ndarray}]."""
    if "runner" in _CACHE:
        return _CACHE["runner"]

    import jax
    from jax.sharding import Mesh, PartitionSpec
    try:
        from jax.experimental.shard_map import shard_map
    except ImportError:  # newer jax
        from jax.sharding import shard_map
    from concourse import mybir
    from concourse.bass2jax import (_bass_exec_p, install_neuronx_cc_hook,
                                    partition_id_tensor)

    nc = _build_nc()
    install_neuronx_cc_hook()

    partition_name = (nc.partition_id_tensor.name
                      if nc.partition_id_tensor else None)
    in_names, out_names, out_avals, zero_outs = [], [], [], []
    for alloc in nc.m.functions[0].allocations:
        if not isinstance(alloc, mybir.MemoryLocationSet):
            continue
        name = alloc.memorylocations[0].name
        if alloc.kind == "ExternalInput":
            if name != partition_name:
                in_names.append(name)
        elif alloc.kind == "ExternalOutput":
            shape = tuple(alloc.tensor_shape)
            dtype = mybir.dt.np(alloc.dtype)
            out_names.append(name)
            out_avals.append(jax.core.ShapedArray(shape, dtype))
            zero_outs.append(np.zeros(shape, dtype))
    n_params = len(in_names)
    n_outs = len(out_avals)
    all_in_names = in_names + out_names + ([partition_name] if partition_name else [])
    donate = tuple(range(n_params, n_params + n_outs))

    def _body(*args):
        operands = list(args)
        if partition_name is not None:
            operands.append(partition_id_tensor())
        return tuple(_bass_exec_p.bind(
            *operands, out_avals=tuple(out_avals), in_names=tuple(all_in_names),
            out_names=tuple(out_names), lowering_input_output_aliases=(),
            sim_require_finite=True, sim_require_nnan=True, nc=nc))

    devices = jax.devices()[:B]
    mesh = Mesh(np.asarray(devices), ("core",))
    sharded = jax.jit(
        shard_map(_body, mesh=mesh,
                  in_specs=(PartitionSpec("core"),) * (n_params + n_outs),
                  out_specs=(PartitionSpec("core"),) * n_outs,
                  check_rep=False),
        donate_argnums=donate, keep_unused=True)

    def run(in_maps):
        per_core = [[np.asarray(m[name]) for name in in_names] for m in in_maps]
        concat_in = [np.concatenate([per_core[c][i] for c in range(B)], axis=0)
                     for i in range(n_params)]
        big_zeros = [np.concatenate([z] * B, axis=0) for z in zero_outs]
        outs = jax.block_until_ready(sharded(*concat_in, *big_zeros))
        results = []
        for c in range(B):
            r = {}
            for i, name in enumerate(out_names):
                rows = out_avals[i].shape[0]
                r[name] = np.asarray(outs[i][c * rows : (c + 1) * rows])
            results.append(r)
        return results

    _CACHE["runner"] = run
    return run


def kernel(**inputs) -> np.ndarray:
    in_maps, x_shape = _prep_inputs(**inputs)
    run = _get_runner()
    results = run(in_maps)
    out = np.stack([results[b]["y"].reshape(x_shape[1], x_shape[2], 1)
                    for b in range(B)])
    return out.astype(np.float32)



# revision 5
# speedup vs baseline: 2.7596x; 2.7596x over previous
"""Trainium2 Bass kernel for the GNO (Galerkin-type linear attention) model.

Reference computation per batch element b (N=4096 tokens, d=64):
    h = x @ lift_w + lift_b
    for each of 4 layers:
        q = h@q_w+q_b ; k = h@k_w+k_b ; v = h@v_w+v_b
        kern     = (q @ k^T) / sqrt(d)          # [N, N], no softmax!
        integral = (kern @ v) / N               # [N, d]
        h        = gelu(h@blk_w+blk_b + integral)
    out = h @ proj_w + proj_b

Because there is no softmax, (q k^T) v == q (k^T v), so each layer only
needs the tiny [64,64] moment matrix ktv = k^T v.  Further,
    integral = q @ (s*ktv)  = H_aug^T (q_w_aug @ (s*ktv))
    wh + integral           = H_aug^T (blk_w_aug + q_w_aug @ (s*ktv))
with H_aug = [h^T; 1] (a ones row folds every bias into the weights), so
the whole layer update is ONE [65,64] combined weight matmul + gelu.

All matmul operands are bf16 (fp32 matmuls are split by the compiler into
2 half-rate passes = 4x cost); PSUM accumulation stays fp32.  Verified
numerically: full bf16 pipeline lands at ~6e-3 rel err vs the fp32
reference (tolerance 2e-2).

Sharding: batch is 2 -> data-parallel on 2 NeuronCores, one batch element
per core, everything resident in SBUF.  Sequence-sharding wider would need
a per-layer AllReduce of ktv whose ~10us latency floor exceeds the whole
per-layer compute, so it loses.
"""

import os
import sys

for _p in ("/opt/trn_rl_repo", "/root/.axon_site/_ro/trn_rl_repo"):
    if os.path.isdir(_p) and _p not in sys.path:
        sys.path.append(_p)

import numpy as np

N = 4096          # tokens per batch element (64*64)
D = 64            # hidden
DA = D + 1        # hidden + ones row
L = 4             # layers
B = 2             # batch / cores used
SCALE = (1.0 / np.sqrt(np.float32(D))) / np.float32(N)

_CACHE = {}


def _build_nc():
    """Build + compile the per-core Bass program (identical on both cores)."""
    import concourse.bass as bass
    import concourse.tile as tile
    from concourse import bacc, mybir

    f32 = mybir.dt.float32
    bf16 = mybir.dt.bfloat16
    ts = bass.ts
    GELU = mybir.ActivationFunctionType.Gelu

    nc = bacc.Bacc("TRN2", target_bir_lowering=False, debug=False, num_devices=B)

    xt_d = nc.dram_tensor("xt", [4, N], bf16, kind="ExternalInput")
    lift_d = nc.dram_tensor("liftw", [4, DA], bf16, kind="ExternalInput")
    kvw_d = nc.dram_tensor("kvw", [DA, L * 2 * D], bf16, kind="ExternalInput")
    qts_d = nc.dram_tensor("qts", [D, L * DA], bf16, kind="ExternalInput")
    blkw_d = nc.dram_tensor("blkw", [DA, L * D], bf16, kind="ExternalInput")
    proj_d = nc.dram_tensor("projw", [DA, 1], bf16, kind="ExternalInput")
    # y is produced token-transposed: y_token(128*q + p) = y_d[p, q]
    y_d = nc.dram_tensor("y", [128, N // 128], f32, kind="ExternalOutput")

    PS = bass.MemorySpace.PSUM

    with tile.TileContext(nc) as tc:
        with (
            tc.tile_pool(name="consts", bufs=1) as consts,
            tc.tile_pool(name="hbuf", bufs=1) as hbuf,
            tc.tile_pool(name="kvsb", bufs=3) as kvsb,
            tc.tile_pool(name="small", bufs=2) as small,
            tc.tile_pool(name="ps_kv", bufs=2, space=PS) as ps_kv,
            tc.tile_pool(name="ps_sm", bufs=2, space=PS) as ps_sm,
            tc.tile_pool(name="ps_up", bufs=2, space=PS) as ps_up,
        ):
            # ---- load everything into SBUF -------------------------------
            xt = consts.tile([4, N], bf16, tag="xt")
            nc.sync.dma_start(xt[:], xt_d.ap())
            liftw = consts.tile([4, DA], bf16, tag="liftw")
            nc.sync.dma_start(liftw[:], lift_d.ap())
            kvw = consts.tile([DA, L * 2 * D], bf16, tag="kvw")
            nc.sync.dma_start(kvw[:], kvw_d.ap())
            qts = consts.tile([D, L * DA], bf16, tag="qts")
            nc.sync.dma_start(qts[:], qts_d.ap())
            blkw = consts.tile([DA, L * D], bf16, tag="blkw")
            nc.sync.dma_start(blkw[:], blkw_d.ap())
            projw = consts.tile([DA, 1], bf16, tag="projw")
            nc.sync.dma_start(projw[:], proj_d.ap())

            # two ping-pong H_aug buffers, [65, 4096] each
            H0 = hbuf.tile([DA, N], bf16, tag="h0")
            H1 = hbuf.tile([DA, N], bf16, tag="h1")
            # H1's ones row never gets written by the layer loop; seed it
            # from the ones row of x^T_aug.  H0's comes from the lift matmul.
            nc.sync.dma_start(H1[D : D + 1, :], xt_d.ap()[3:4, :])

            # ---- lift: H0 = lift_aug^T @ xt  ([65, 4096]) ----------------
            for g in range(8):
                ps = ps_kv.tile([DA, 512], f32, tag="kv")
                nc.tensor.matmul(ps[:], liftw[:], xt[:, ts(g, 512)],
                                 start=True, stop=True)
                nc.vector.tensor_copy(H0[:, ts(g, 512)], ps[:])

            # ---- layers --------------------------------------------------
            for l in range(L):
                cur = H0 if l % 2 == 0 else H1
                nxt = H1 if l % 2 == 0 else H0
                kvw_l = kvw[:, l * 2 * D : (l + 1) * 2 * D]

                ktv_ps = ps_sm.tile([D, D], f32, tag="sm")
                for g in range(8):
                    kv_ps = ps_kv.tile([128, 512], f32, tag="kv")
                    for k in range(4):
                        t = 4 * g + k
                        # KV tile [128, 128] = H_chunk^T @ kvw_aug
                        nc.tensor.matmul(kv_ps[:, ts(k, 128)],
                                         cur[:, ts(t, 128)], kvw_l,
                                         start=True, stop=True)
                    kv_sb = kvsb.tile([128, 512], bf16, tag="kvt")
                    nc.vector.tensor_copy(kv_sb[:], kv_ps[:])
                    for k in range(4):
                        first = (g == 0 and k == 0)
                        last = (g == 7 and k == 3)
                        # ktv += K_tile^T @ V_tile (fp32 accumulation)
                        nc.tensor.matmul(ktv_ps[:],
                                         kv_sb[:, k * 128 : k * 128 + 64],
                                         kv_sb[:, k * 128 + 64 : k * 128 + 128],
                                         start=first, stop=last)

                ktv_sb = small.tile([D, D], bf16, tag="ktv")
                nc.vector.tensor_copy(ktv_sb[:], ktv_ps[:])

                # W_upd = blk_w_aug + q_w_aug*s @ ktv   ([65, 64])
                weff_ps = ps_sm.tile([DA, D], f32, tag="sm")
                nc.tensor.matmul(weff_ps[:], qts[:, l * DA : (l + 1) * DA],
                                 ktv_sb[:], start=True, stop=True)
                wupd_sb = small.tile([DA, D], bf16, tag="wupd")
                nc.vector.tensor_add(wupd_sb[:], weff_ps[:],
                                     blkw[:, l * D : (l + 1) * D])

                # h' = gelu(H_aug^T @ W_upd), written transposed into nxt
                for c in range(4):
                    up_ps = ps_up.tile([D, 1024], f32, tag="up")
                    for i in range(2):
                        nc.tensor.matmul(
                            up_ps[:, ts(i, 512)], wupd_sb[:],
                            cur[:, 1024 * c + 512 * i : 1024 * c + 512 * (i + 1)],
                            start=True, stop=True)
                    nc.scalar.activation(nxt[0:D, ts(c, 1024)], up_ps[:], GELU)

            # ---- proj: y^T = H_final^T @ proj_aug, token-transposed ------
            # yT[p, q] = y_token(128*q + p); one [128, 32] psum bank, one copy.
            Hf = H0 if L % 2 == 0 else H1
            yt_full = ps_sm.tile([128, D], f32, tag="sm")
            yt_ps = yt_full[:, 0 : N // 128]
            for q in range(N // 128):
                nc.tensor.matmul(yt_ps[:, q : q + 1], Hf[:, ts(q, 128)],
                                 projw[:], start=True, stop=True)
            out_sb = consts.tile([128, N // 128], f32, tag="out")
            nc.vector.tensor_copy(out_sb[:], yt_ps[:])
            nc.sync.dma_start(y_d.ap(), out_sb[:])

    nc.compile()
    return nc


def _prep_inputs(x, lift_w, lift_b, blk_w, blk_b, q_w, q_b, k_w, k_b, v_w,
                 v_b, proj_w, proj_b):
    """Host-side weight packing (tiny [64,64] reshuffles, negligible cost)."""
    from ml_dtypes import bfloat16

    f = lambda a: np.asarray(a, dtype=np.float32)
    bf = lambda a: np.ascontiguousarray(np.asarray(a, np.float32),
                                        dtype=np.float32).astype(bfloat16)
    x = f(x)
    lift_w, lift_b = f(lift_w), f(lift_b)
    blk_w, blk_b = f(blk_w), f(blk_b)
    q_w, q_b, k_w, k_b, v_w, v_b = f(q_w), f(q_b), f(k_w), f(k_b), f(v_w), f(v_b)
    proj_w, proj_b = f(proj_w), f(proj_b)

    lift_aug = np.zeros((4, DA), np.float32)
    lift_aug[:3, :D] = lift_w
    lift_aug[3, :D] = lift_b
    lift_aug[3, D] = 1.0  # makes the lift matmul emit H0's ones row

    kvw = np.concatenate(
        [np.concatenate([np.vstack([k_w[l], k_b[l][None]]),
                         np.vstack([v_w[l], v_b[l][None]])], axis=1)
         for l in range(L)], axis=1).astype(np.float32)          # [65, 512]
    qts = np.concatenate(
        [(np.vstack([q_w[l], q_b[l][None]]) * SCALE).T
         for l in range(L)], axis=1).astype(np.float32)          # [64, 260]
    blkw = np.concatenate(
        [np.vstack([blk_w[l], blk_b[l][None]]) for l in range(L)],
        axis=1).astype(np.float32)                               # [65, 256]
    proj = np.vstack([proj_w, proj_b[None]]).astype(np.float32)  # [65, 1]

    in_maps = []
    for b in range(B):
        xt = np.concatenate([x[b].reshape(N, 3).T,
                             np.ones((1, N), np.float32)], axis=0)
        in_maps.append({"xt": bf(xt), "liftw": bf(lift_aug),
                        "kvw": bf(kvw), "qts": bf(qts), "blkw": bf(blkw),
                        "projw": bf(proj)})
    return in_maps, x.shape


def _unpack_y(y_np):
    """[128, 32] token-transposed fp32 -> flat [4096] token order."""
    return np.ascontiguousarray(np.asarray(y_np, np.float32).T).reshape(N)


def _get_runner():
    """Compile once, return a fn(in_maps) -> list[{name: np.ndarray}]."""
    if "runner" in _CACHE:
        return _CACHE["runner"]

    import jax
    from jax.sharding import Mesh, PartitionSpec
    try:
        from jax.experimental.shard_map import shard_map
    except ImportError:  # newer jax
        from jax.sharding import shard_map
    from concourse import mybir
    from concourse.bass2jax import (_bass_exec_p, install_neuronx_cc_hook,
                                    partition_id_tensor)

    nc = _build_nc()
    install_neuronx_cc_hook()

    partition_name = (nc.partition_id_tensor.name
                      if nc.partition_id_tensor else None)
    in_names, out_names, out_avals, zero_outs = [], [], [], []
    for alloc in nc.m.functions[0].allocations:
        if not isinstance(alloc, mybir.MemoryLocationSet):
            continue
        name = alloc.memorylocations[0].name
        if alloc.kind == "ExternalInput":
            if name != partition_name:
                in_names.append(name)
        elif alloc.kind == "ExternalOutput":
            shape = tuple(alloc.tensor_shape)
            dtype = mybir.dt.np(alloc.dtype)
            out_names.append(name)
            out_avals.append(jax.core.ShapedArray(shape, dtype))
            zero_outs.append(np.zeros(shape, dtype))
    n_params = len(in_names)
    n_outs = len(out_avals)
    all_in_names = in_names + out_names + ([partition_name] if partition_name else [])
    donate = tuple(range(n_params, n_params + n_outs))

    def _body(*args):
        operands = list(args)
        if partition_name is not None:
            operands.append(partition_id_tensor())
        return tuple(_bass_exec_p.bind(
            *operands, out_avals=tuple(out_avals), in_names=tuple(all_in_names),
            out_names=tuple(out_names), lowering_input_output_aliases=(),
            sim_require_finite=True, sim_require_nnan=True, nc=nc))

    devices = jax.devices()[:B]
    mesh = Mesh(np.asarray(devices), ("core",))
    sharded = jax.jit(
        shard_map(_body, mesh=mesh,
                  in_specs=(PartitionSpec("core"),) * (n_params + n_outs),
                  out_specs=(PartitionSpec("core"),) * n_outs,
                  check_rep=False),
        donate_argnums=donate, keep_unused=True)

    def run(in_maps):
        per_core = [[np.asarray(m[name]) for name in in_names] for m in in_maps]
        concat_in = [np.concatenate([per_core[c][i] for c in range(B)], axis=0)
                     for i in range(n_params)]
        big_zeros = [np.concatenate([z] * B, axis=0) for z in zero_outs]
        outs = jax.block_until_ready(sharded(*concat_in, *big_zeros))
        results = []
        for c in range(B):
            r = {}
            for i, name in enumerate(out_names):
                rows = out_avals[i].shape[0]
                r[name] = np.asarray(outs[i][c * rows : (c + 1) * rows])
            results.append(r)
        return results

    _CACHE["runner"] = run
    return run


def kernel(**inputs) -> np.ndarray:
    in_maps, x_shape = _prep_inputs(**inputs)
    run = _get_runner()
    results = run(in_maps)
    out = np.stack([_unpack_y(results[b]["y"]).reshape(x_shape[1], x_shape[2], 1)
                    for b in range(B)])
    return out.astype(np.float32)


# revision 13
# speedup vs baseline: 2.9152x; 1.0564x over previous
"""Trainium2 Bass kernel for the GNO (Galerkin-type linear attention) model.

Reference computation per batch element b (N=4096 tokens, d=64):
    h = x @ lift_w + lift_b
    for each of 4 layers:
        q = h@q_w+q_b ; k = h@k_w+k_b ; v = h@v_w+v_b
        kern     = (q @ k^T) / sqrt(d)          # [N, N], no softmax!
        integral = (kern @ v) / N               # [N, d]
        h        = gelu(h@blk_w+blk_b + integral)
    out = h @ proj_w + proj_b

Because there is no softmax, (q k^T) v == q (k^T v), so each layer only
needs the tiny [64,64] moment matrix ktv = k^T v.  Further,
    integral = q @ (s*ktv)  = H_aug^T (q_w_aug @ (s*ktv))
    wh + integral           = H_aug^T (blk_w_aug + q_w_aug @ (s*ktv))
with H_aug = [h^T; 1] (a ones row folds every bias into the weights), so
the whole layer update is ONE [65,64] combined weight matmul + gelu.

All matmul operands are bf16 (fp32 matmuls are split by the compiler into
2 half-rate passes = 4x cost); PSUM accumulation stays fp32.  Verified
numerically: full bf16 pipeline lands at ~6e-3 rel err vs the fp32
reference (tolerance 2e-2).

Sharding: batch is 2 -> data-parallel on 2 NeuronCores, one batch element
per core, everything resident in SBUF.  Sequence-sharding wider would need
a per-layer AllReduce of ktv whose ~10us latency floor exceeds the whole
per-layer compute, so it loses.
"""

import os
import sys

for _p in ("/opt/trn_rl_repo", "/root/.axon_site/_ro/trn_rl_repo"):
    if os.path.isdir(_p) and _p not in sys.path:
        sys.path.append(_p)

import numpy as np

N = 4096          # tokens per batch element (64*64)
D = 64            # hidden
DA = D + 1        # hidden + ones row
L = 4             # layers
B = 2             # batch / cores used
SCALE = (1.0 / np.sqrt(np.float32(D))) / np.float32(N)

_CACHE = {}


def _build_nc():
    """Build + compile the per-core Bass program (identical on both cores)."""
    import concourse.bass as bass
    import concourse.tile as tile
    from concourse import bacc, mybir

    f32 = mybir.dt.float32
    bf16 = mybir.dt.bfloat16
    ts = bass.ts
    GELU = mybir.ActivationFunctionType.Gelu
    COPY = mybir.ActivationFunctionType.Copy

    nc = bacc.Bacc("TRN2", target_bir_lowering=False, debug=False, num_devices=B)

    xt_d = nc.dram_tensor("xt", [4, N], bf16, kind="ExternalInput")
    lift_d = nc.dram_tensor("liftw", [4, DA], bf16, kind="ExternalInput")
    # kvw zero-padded to K=128 so chunk LDWEIGHTS are FWL-eligible
    kvw_d = nc.dram_tensor("kvw", [128, L * 2 * D], bf16, kind="ExternalInput")
    qts_d = nc.dram_tensor("qts", [D, L * DA], bf16, kind="ExternalInput")
    blkw_d = nc.dram_tensor("blkw", [DA, L * D], bf16, kind="ExternalInput")
    proj_d = nc.dram_tensor("projw", [128, 1], bf16, kind="ExternalInput")
    ident_d = nc.dram_tensor("ident", [DA, DA], bf16, kind="ExternalInput")
    # y is produced token-transposed: y_token(128*q + p) = y_d[p, q]
    y_d = nc.dram_tensor("y", [128, N // 128], f32, kind="ExternalOutput")

    PS = bass.MemorySpace.PSUM

    with tile.TileContext(nc) as tc:
        with (
            tc.tile_pool(name="consts", bufs=1) as consts,
            tc.tile_pool(name="hbuf", bufs=1) as hbuf,
            tc.tile_pool(name="kvsb", bufs=3) as kvsb,
            tc.tile_pool(name="small", bufs=2) as small,
            tc.tile_pool(name="ps_kv", bufs=2, space=PS) as ps_kv,
            tc.tile_pool(name="ps_sm", bufs=2, space=PS) as ps_sm,
            tc.tile_pool(name="ps_up", bufs=2, space=PS) as ps_up,
        ):
            # ---- load everything into SBUF -------------------------------
            xt = consts.tile([4, N], bf16, tag="xt")
            nc.sync.dma_start(xt[:], xt_d.ap())
            liftw = consts.tile([4, DA], bf16, tag="liftw")
            nc.sync.dma_start(liftw[:], lift_d.ap())
            kvw = consts.tile([128, L * 2 * D], bf16, tag="kvw")
            nc.sync.dma_start(kvw[:], kvw_d.ap())
            qts = consts.tile([D, L * DA], bf16, tag="qts")
            nc.sync.dma_start(qts[:], qts_d.ap())
            blkw = consts.tile([DA, L * D], bf16, tag="blkw")
            nc.sync.dma_start(blkw[:], blkw_d.ap())
            projw = consts.tile([128, 1], bf16, tag="projw")
            nc.sync.dma_start(projw[:], proj_d.ap())
            ident = consts.tile([DA, DA], bf16, tag="ident")
            nc.sync.dma_start(ident[:], ident_d.ap())

            # H buffers padded to 128 partitions: rows 0-63 h, row 64 ones,
            # rows 65-127 zero (so [128,128] chunk LDWEIGHTS can use FWL).
            H0 = hbuf.tile([128, N], bf16, tag="h0")
            H1 = hbuf.tile([128, N], bf16, tag="h1")
            # zero rows 64-127 first; the ones row (64) is then written on
            # top by the lift matmul (H0) / the DMA below (H1).
            nc.gpsimd.memset(H0[D:128, :], 0.0)
            nc.gpsimd.memset(H1[D:128, :], 0.0)
            # H1's ones row never gets written by the layer loop; seed it
            # from the ones row of x^T_aug.  H0's comes from the lift matmul.
            nc.sync.dma_start(H1[D : D + 1, :], xt_d.ap()[3:4, :])

            # ---- lift: H0 = lift_aug^T @ xt  ([65, 4096]) ----------------
            for g in range(8):
                ps = ps_kv.tile([128, 512], f32, tag="kv")
                nc.tensor.matmul(ps[0:DA, :], liftw[:], xt[:, ts(g, 512)],
                                 start=True, stop=True)
                if g % 2 == 0:
                    nc.vector.tensor_copy(H0[0:DA, ts(g, 512)], ps[0:DA, :])
                else:
                    nc.scalar.activation(H0[0:DA, ts(g, 512)], ps[0:DA, :], COPY)

            # ---- layers --------------------------------------------------
            # software-pipelined: KV(g+1) is emitted before ktv(g) so the
            # tensor engine never sits waiting on the PSUM->SBUF copy of g.
            for l in range(L):
                cur = H0 if l % 2 == 0 else H1
                nxt = H1 if l % 2 == 0 else H0
                kvw_l = kvw[:, l * 2 * D : (l + 1) * 2 * D]

                ktv_ps = ps_sm.tile([128, D], f32, tag="sm")
                kv_sbs = [None] * 8

                def kv_group(g):
                    kv_ps = ps_kv.tile([128, 512], f32, tag="kv")
                    for k in range(4):
                        t = 4 * g + k
                        # KV tile [128, 128] = H_chunk^T @ kvw_aug (K=128 padded)
                        nc.tensor.matmul(kv_ps[:, ts(k, 128)],
                                         cur[:, ts(t, 128)], kvw_l,
                                         start=True, stop=True)
                    kv_sb = kvsb.tile([128, 512], bf16, tag="kvt")
                    if g % 4 == 3:
                        nc.scalar.activation(kv_sb[:], kv_ps[:], COPY)
                    else:
                        nc.vector.tensor_copy(kv_sb[:], kv_ps[:])
                    kv_sbs[g] = kv_sb

                def ktv_group(g):
                    kv_sb = kv_sbs[g]
                    for k in range(4):
                        first = (g == 0 and k == 0)
                        last = (g == 7 and k == 3)
                        # ktv += KV_tile^T @ V_tile; lhsT is the full 128-col
                        # KV tile (FWL), rows 64:128 of the product (V^T V)
                        # land in psum rows 64:128 and are never read.
                        nc.tensor.matmul(ktv_ps[:],
                                         kv_sb[:, k * 128 : (k + 1) * 128],
                                         kv_sb[:, k * 128 + 64 : k * 128 + 128],
                                         start=first, stop=last)

                kv_group(0)
                for g in range(1, 8):
                    kv_group(g)
                    ktv_group(g - 1)
                ktv_group(7)

                ktv_sb = small.tile([D, D], bf16, tag="ktv")
                nc.vector.tensor_copy(ktv_sb[:], ktv_ps[0:D, :])

                # W_upd = blk_w_aug + q_w_aug*s @ ktv   ([65, 64])
                weff_ps = ps_sm.tile([DA, D], f32, tag="sm")
                nc.tensor.matmul(weff_ps[:], qts[:, l * DA : (l + 1) * DA],
                                 ktv_sb[:], start=True, stop=False)
                # += blkw via PE (identity stationary) instead of a DVE add
                nc.tensor.matmul(weff_ps[:], ident[:],
                                 blkw[:, l * D : (l + 1) * D],
                                 start=False, stop=True)
                wupd_sb = small.tile([DA, D], bf16, tag="wupd")
                nc.vector.tensor_copy(wupd_sb[:], weff_ps[:])

                # h' = gelu(H_aug^T @ W_upd), written transposed into nxt
                for c in range(4):
                    up_ps = ps_up.tile([D, 1024], f32, tag="up")
                    for i in range(2):
                        nc.tensor.matmul(
                            up_ps[:, ts(i, 512)], wupd_sb[:],
                            cur[0:DA, 1024 * c + 512 * i : 1024 * c + 512 * (i + 1)],
                            start=True, stop=True)
                    nc.scalar.activation(nxt[0:D, ts(c, 1024)], up_ps[:], GELU)

            # ---- proj: y^T = H_final^T @ proj_aug, token-transposed ------
            # yT[p, q] = y_token(128*q + p); one [128, 32] psum bank, one copy.
            Hf = H0 if L % 2 == 0 else H1
            yt_full = ps_sm.tile([128, D], f32, tag="sm")
            yt_ps = yt_full[:, 0 : N // 128]
            for q in range(N // 128):
                nc.tensor.matmul(yt_ps[:, q : q + 1], Hf[:, ts(q, 128)],
                                 projw[:], start=True, stop=True)
            out_sb = consts.tile([128, N // 128], f32, tag="out")
            nc.vector.tensor_copy(out_sb[:], yt_ps[:])
            nc.sync.dma_start(y_d.ap(), out_sb[:])

    nc.compile()
    return nc


def _prep_inputs(x, lift_w, lift_b, blk_w, blk_b, q_w, q_b, k_w, k_b, v_w,
                 v_b, proj_w, proj_b):
    """Host-side weight packing (tiny [64,64] reshuffles, negligible cost)."""
    from ml_dtypes import bfloat16

    f = lambda a: np.asarray(a, dtype=np.float32)
    bf = lambda a: np.ascontiguousarray(np.asarray(a, np.float32),
                                        dtype=np.float32).astype(bfloat16)
    x = f(x)
    lift_w, lift_b = f(lift_w), f(lift_b)
    blk_w, blk_b = f(blk_w), f(blk_b)
    q_w, q_b, k_w, k_b, v_w, v_b = f(q_w), f(q_b), f(k_w), f(k_b), f(v_w), f(v_b)
    proj_w, proj_b = f(proj_w), f(proj_b)

    lift_aug = np.zeros((4, DA), np.float32)
    lift_aug[:3, :D] = lift_w
    lift_aug[3, :D] = lift_b
    lift_aug[3, D] = 1.0  # makes the lift matmul emit H0's ones row

    kvw = np.zeros((128, L * 2 * D), np.float32)                 # K=128 padded
    for l in range(L):
        kvw[:DA, l * 2 * D : l * 2 * D + D] = np.vstack([k_w[l], k_b[l][None]])
        kvw[:DA, l * 2 * D + D : (l + 1) * 2 * D] = np.vstack([v_w[l], v_b[l][None]])
    qts = np.concatenate(
        [(np.vstack([q_w[l], q_b[l][None]]) * SCALE).T
         for l in range(L)], axis=1).astype(np.float32)          # [64, 260]
    blkw = np.concatenate(
        [np.vstack([blk_w[l], blk_b[l][None]]) for l in range(L)],
        axis=1).astype(np.float32)                               # [65, 256]
    proj = np.zeros((128, 1), np.float32)                        # K=128 padded
    proj[:DA, 0] = np.concatenate([proj_w[:, 0], proj_b])
    ident = np.eye(DA, dtype=np.float32)                         # [65, 65]

    in_maps = []
    for b in range(B):
        xt = np.concatenate([x[b].reshape(N, 3).T,
                             np.ones((1, N), np.float32)], axis=0)
        in_maps.append({"xt": bf(xt), "liftw": bf(lift_aug),
                        "kvw": bf(kvw), "qts": bf(qts), "blkw": bf(blkw),
                        "projw": bf(proj), "ident": bf(ident)})
    return in_maps, x.shape


def _unpack_y(y_np):
    """[128, 32] token-transposed fp32 -> flat [4096] token order."""
    return np.ascontiguousarray(np.asarray(y_np, np.float32).T).reshape(N)


def _get_runner():
    """Compile once, return a fn(in_maps) -> list[{name: np.ndarray}]."""
    if "runner" in _CACHE:
        return _CACHE["runner"]

    import jax
    from jax.sharding import Mesh, PartitionSpec
    try:
        from jax.experimental.shard_map import shard_map
    except ImportError:  # newer jax
        from jax.sharding import shard_map
    from concourse import mybir
    from concourse.bass2jax import (_bass_exec_p, install_neuronx_cc_hook,
                                    partition_id_tensor)

    nc = _build_nc()
    install_neuronx_cc_hook()

    partition_name = (nc.partition_id_tensor.name
                      if nc.partition_id_tensor else None)
    in_names, out_names, out_avals, zero_outs = [], [], [], []
    for alloc in nc.m.functions[0].allocations:
        if not isinstance(alloc, mybir.MemoryLocationSet):
            continue
        name = alloc.memorylocations[0].name
        if alloc.kind == "ExternalInput":
            if name != partition_name:
                in_names.append(name)
        elif alloc.kind == "ExternalOutput":
            shape = tuple(alloc.tensor_shape)
            dtype = mybir.dt.np(alloc.dtype)
            out_names.append(name)
            out_avals.append(jax.core.ShapedArray(shape, dtype))
            zero_outs.append(np.zeros(shape, dtype))
    n_params = len(in_names)
    n_outs = len(out_avals)
    all_in_names = in_names + out_names + ([partition_name] if partition_name else [])
    donate = tuple(range(n_params, n_params + n_outs))

    def _body(*args):
        operands = list(args)
        if partition_name is not None:
            operands.append(partition_id_tensor())
        return tuple(_bass_exec_p.bind(
            *operands, out_avals=tuple(out_avals), in_names=tuple(all_in_names),
            out_names=tuple(out_names), lowering_input_output_aliases=(),
            sim_require_finite=True, sim_require_nnan=True, nc=nc))

    devices = jax.devices()[:B]
    mesh = Mesh(np.asarray(devices), ("core",))
    sharded = jax.jit(
        shard_map(_body, mesh=mesh,
                  in_specs=(PartitionSpec("core"),) * (n_params + n_outs),
                  out_specs=(PartitionSpec("core"),) * n_outs,
                  check_rep=False),
        donate_argnums=donate, keep_unused=True)

    def run(in_maps):
        per_core = [[np.asarray(m[name]) for name in in_names] for m in in_maps]
        concat_in = [np.concatenate([per_core[c][i] for c in range(B)], axis=0)
                     for i in range(n_params)]
        big_zeros = [np.concatenate([z] * B, axis=0) for z in zero_outs]
        outs = jax.block_until_ready(sharded(*concat_in, *big_zeros))
        results = []
        for c in range(B):
            r = {}
            for i, name in enumerate(out_names):
                rows = out_avals[i].shape[0]
                r[name] = np.asarray(outs[i][c * rows : (c + 1) * rows])
            results.append(r)
        return results

    _CACHE["runner"] = run
    return run


def kernel(**inputs) -> np.ndarray:
    in_maps, x_shape = _prep_inputs(**inputs)
    run = _get_runner()
    results = run(in_maps)
    out = np.stack([_unpack_y(results[b]["y"]).reshape(x_shape[1], x_shape[2], 1)
                    for b in range(B)])
    return out.astype(np.float32)
